# revision 7
# baseline (speedup 1.0000x reference)
"""ConcatenatedLoRALinearSidecarLayer kernel for 8x TRN2 NeuronCores.

Reference computation (per LoRA branch n, then concat over n on the last dim):
    h_n = x @ down_n.T                      # [M, R]
    y_n = (h_n @ up_n.T + bias_n) * (WEIGHT * scales_n)
    out = concat_n(y_n)                     # [M, N*O]

Strategy:
  - Data-parallel over tokens M = B*S = 16384 -> 2048 tokens per core.
  - Host-side prep: transpose x to [D, M] so the device only ever does
    contiguous DMAs; fold WEIGHT*scales into up and bias.
  - Per core, for each 256-token block:
      phase 1:  hT_n[r, t] = sum_d downT_n[d, r] * xT[d, t]
                (downT stationary, xT moving with free dim 256 -> fp32r runs
                 at 1 cycle/row on the PE)
      phase 2:  y[t, o] = sum_r hT_n[r, t] * upT_n[r, o]
                (hT block stationary, upT moving with free dim 512)
      copyback: DVE adds the (pre-scaled) bias during PSUM->SBUF copy.
  - All weights (downT, upT, bias) stay resident in SBUF.

Wait-slot legalization: this container's walrus accepts at most 1 sync-wait
on a matmul and ~2 on other instructions. The kernel is structured so every
matmul has deps on at most ONE other processor (DVE or one DMA lane):
  - tiny DVE "funnel" copies absorb DMA-completion deps for const tiles,
  - a tiny PE matmul at start absorbs the DVE funnel tick into PE's clock,
  - PSUM pool depths chosen so slot-release deps are covered by older waits,
  - tiny DVE funnel after each output DMA so SBUF slot releases reach DVE.
The Tile tail drain (which waits on every semaphore) is split into a chain
of NoOps carrying <=2 waits each via a TileContext subclass.
"""

from contextlib import ExitStack

import numpy as np

import concourse.bass as bass
import concourse.mybir as mybir
import concourse.tile as tile
from concourse.vector_clock import ScopedClock

WEIGHT = 0.8
N_CORES = 8
B, S, D = 4, 4096, 4096
NL, R, O = 3, 128, 4096
M = B * S                    # 16384 tokens total
T = M // N_CORES             # 2048 tokens per core
NR = NL * R                  # 384
NO = NL * O                  # 12288

P = 128                      # SBUF partitions
TB = 256                     # token block (phase-1 moving free dim)
DO = D // P                  # 32 contraction chunks
OC = 512                     # phase-2 moving free dim / PSUM tile

F32 = mybir.dt.float32
F32R = mybir.dt.float32r

MAX_DRAIN_WAITS = 1


class SplitDrainTileContext(tile.TileContext):
    """TileContext whose tail drain splits its waits into <=2 per instruction.

    The stock tail emits one SP Drain carrying a wait for every live
    semaphore; this walrus build rejects >2 sync-waits per instruction.
    Semantics are identical: all waits still complete before the all-engine
    barrier that follows.
    """

    def _drain_and_barrier(self, tick_clock, wait_clock):
        drain_inst = self.nc.sync.drain()
        wait_clock.add_sem_waits(
            drain_inst.ins, ScopedClock({None: tick_clock.global_clock})
        )
        si = drain_inst.ins.sync_info
        if si is not None and len(si.on_wait) > MAX_DRAIN_WAITS:
            waits = list(si.on_wait)
            drain_inst.ins.sync_info = mybir.SyncInfo(
                on_wait=waits[:MAX_DRAIN_WAITS], on_update=list(si.on_update)
            )
            for i in range(MAX_DRAIN_WAITS, len(waits), MAX_DRAIN_WAITS):
                nop = self.nc.sync.nop(nofuse=True)
                nop.ins.sync_info = mybir.SyncInfo(
                    on_wait=waits[i:i + MAX_DRAIN_WAITS], on_update=[]
                )

        self.nc.all_engine_barrier()
        popped = self.nc._tile_sem_poison_stack.pop()
        assert popped is self._sem_poison
        self.nc.clear_and_free_semaphores(list(self.sems.allocated().values()))
        self.nc.all_engine_barrier()


def build_nc(t_core: int = T) -> bass.Bass:
    assert t_core % TB == 0
    n_tb = t_core // TB

    nc = bass.Bass("TRN2", target_bir_lowering=False, debug=False)

    xT = nc.dram_tensor("xT", [D, t_core], F32R, kind="ExternalInput")
    dT = nc.dram_tensor("dT", [D, NR], F32R, kind="ExternalInput")
    uT = nc.dram_tensor("uT", [R, NO], F32R, kind="ExternalInput")
    bw = nc.dram_tensor("bw", [1, NO], F32, kind="ExternalInput")
    y = nc.dram_tensor("y", [t_core, NO], F32, kind="ExternalOutput")

    with tile.TileContext(nc) as tc, ExitStack() as ctx:
        const = ctx.enter_context(tc.tile_pool(name="const", bufs=1))
        xpool = ctx.enter_context(tc.tile_pool(name="xpool", bufs=2))
        hpool = ctx.enter_context(tc.tile_pool(name="hpool", bufs=2))
        ypool = ctx.enter_context(tc.tile_pool(name="ypool", bufs=3))
        ps_h = ctx.enter_context(tc.tile_pool(name="ps_h", bufs=6, space="PSUM"))
        ps_y = ctx.enter_context(tc.tile_pool(name="ps_y", bufs=2, space="PSUM"))

        # Resident weights
        dT_sb = const.tile([P, DO, NR], F32R, name="dT_sb")
        nc.sync.dma_start(dT_sb[:], dT.ap().rearrange("(do di) nr -> di do nr", di=P))
        uT_sb = const.tile([P, NO], F32R, name="uT_sb")
        nc.sync.dma_start(uT_sb[:], uT[:, :])
        bw_sb = const.tile([P, NO], F32, name="bw_sb")
        nc.sync.dma_start(bw_sb[:], bw.ap().to_broadcast((P, NO)))

        xTr = xT.ap().rearrange("(do di) t -> di do t", di=P)
        DH = DO // 2  # d-chunks per x half-load

        for tb in range(n_tb):
            # Load this block's x slice in two halves so MMs start early.
            xts = []
            for h in range(2):
                xt = xpool.tile([P, DH, TB], F32R, tag="xt", name=f"xt{tb}_{h}")
                nc.sync.dma_start(
                    xt[:], xTr[:, h * DH:(h + 1) * DH, tb * TB:(tb + 1) * TB]
                )
                xts.append(xt)

            # Phase 1: hT_n[r, 0:TB] accumulated over all d chunks.
            hps = [
                ps_h.tile([P, TB], F32, tag="hps", name=f"hps{tb}_{n}")
                for n in range(NL)
            ]
            for dc in range(DO):
                xs = xts[dc // DH][:, dc % DH, :]
                for n in range(NL):
                    nc.tensor.matmul(
                        hps[n][:],
                        dT_sb[:, dc, n * R:(n + 1) * R],
                        xs,
                        start=(dc == 0),
                        stop=(dc == DO - 1),
                    )

            hT = hpool.tile([P, NL, TB], F32R, tag="hT", name=f"hT{tb}")
            for n in range(NL):
                nc.vector.tensor_copy(hT[:, n, :], hps[n][:])

            # Phase 2: y[t, o] per 128-token sub-block, per branch, per o half.
            for th in range(TB // P):
                t0 = tb * TB + th * P
                lhs = [hT[:, n, th * P:(th + 1) * P] for n in range(NL)]
                for n in range(NL):
                    for oh in range(2):
                        o0 = n * O + oh * (O // 2)
                        ysb = ypool.tile([P, O // 2], F32, tag="ysb",
                                         name=f"ysb{tb}_{th}_{n}_{oh}")
                        for oc in range(O // 2 // OC):
                            yps = ps_y.tile([P, OC], F32, tag="yps",
                                            name=f"yps{tb}_{th}_{n}_{oh}_{oc}")
                            nc.tensor.matmul(
                                yps[:],
                                lhs[n],
                                uT_sb[:, o0 + oc * OC: o0 + (oc + 1) * OC],
                                start=True,
                                stop=True,
                            )
                            nc.vector.tensor_add(
                                ysb[:, oc * OC:(oc + 1) * OC],
                                yps[:],
                                bw_sb[:, o0 + oc * OC: o0 + (oc + 1) * OC],
                            )
                        nc.sync.dma_start(
                            y[t0:t0 + P, o0: o0 + O // 2], ysb[:]
                        )

    _wrap_to_json_with_wait_split(nc)
    return nc


def _legalize_wait_counts(bir: dict) -> None:
    """Split multi-wait instructions: this walrus accepts only ONE sync-wait
    per instruction. Excess waits move onto NoOps inserted just before the
    instruction on the same engine — identical blocking semantics."""
    n_new = 0
    for fn in bir.get("functions", []):
        for blk in fn.get("blocks", []):
            insts = blk.get("instructions", [])
            out = []
            for inst in insts:
                si = inst.get("sync_info")
                waits = (si or {}).get("on_wait") or []
                if len(waits) > 1:
                    for w in waits[:-1]:
                        nonlocal_name = f"I-waitsplit-{id(inst)}-{n_new}"
                        n_new += 1
                        out.append({
                            "debug": inst.get("debug", 0),
                            "engine": inst["engine"],
                            "ins": [],
                            "name": nonlocal_name,
                            "opcode": "NoOp",
                            "outs": [],
                            "sync_info": {"on_update": [], "on_wait": [w]},
                        })
                    si["on_wait"] = [waits[-1]]
                out.append(inst)
            blk["instructions"] = out


def _wrap_to_json_with_wait_split(nc) -> None:
    import json as _json

    orig = nc.to_json_bytes

    def patched():
        d = _json.loads(orig())
        _legalize_wait_counts(d)
        return _json.dumps(d).encode()

    nc.to_json_bytes = patched


def prep_inputs(x, down, up, bias, scales):
    """Host-side marshalling: transpose + fold scales. Returns per-core in_maps."""
    x = np.asarray(x, dtype=np.float32)
    down = np.asarray(down, dtype=np.float32)
    up = np.asarray(up, dtype=np.float32)
    bias = np.asarray(bias, dtype=np.float32)
    scales = np.asarray(scales, dtype=np.float32)

    ws = (WEIGHT * scales).astype(np.float32)                       # [NL]
    xTf = np.ascontiguousarray(x.reshape(M, D).T)                   # [D, M]
    dTf = np.ascontiguousarray(np.transpose(down, (2, 0, 1)).reshape(D, NR))
    uTf = np.ascontiguousarray(
        np.transpose(up * ws[:, None, None], (2, 0, 1)).reshape(R, NO)
    ).astype(np.float32)
    bwf = np.ascontiguousarray((bias * ws[:, None]).reshape(1, NO)).astype(np.float32)

    in_maps = []
    for c in range(N_CORES):
        in_maps.append({
            "xT": np.ascontiguousarray(xTf[:, c * T:(c + 1) * T]),
            "dT": dTf,
            "uT": uTf,
            "bw": bwf,
        })
    return in_maps


_CACHED_NC = None


def kernel(x, down, up, bias, scales):
    global _CACHED_NC
    from concourse.bass_utils import run_bass_kernel_spmd

    in_maps = prep_inputs(x, down, up, bias, scales)
    if _CACHED_NC is None:
        _CACHED_NC = build_nc(T)
    res = run_bass_kernel_spmd(_CACHED_NC, in_maps, core_ids=list(range(N_CORES)))
    out = np.concatenate([r["y"] for r in res.results], axis=0)
    return out.reshape(B, S, NO)


# revision 8
# speedup vs baseline: 1.0595x; 1.0595x over previous
"""ConcatenatedLoRALinearSidecarLayer kernel for 8x TRN2 NeuronCores.

Reference computation (per LoRA branch n, then concat over n on the last dim):
    h_n = x @ down_n.T                      # [M, R]
    y_n = (h_n @ up_n.T + bias_n) * (WEIGHT * scales_n)
    out = concat_n(y_n)                     # [M, N*O]

Strategy:
  - Data-parallel over tokens M = B*S = 16384 -> 2048 tokens per core.
  - Host-side prep: transpose x to [D, M] so the device only ever does
    contiguous DMAs; fold WEIGHT*scales into up and bias.
  - Per core, for each 256-token block:
      phase 1:  hT_n[r, t] = sum_d downT_n[d, r] * xT[d, t]
                (downT stationary, xT moving with free dim 256 -> fp32r runs
                 at 1 cycle/row on the PE)
      phase 2:  y[t, o] = sum_r hT_n[r, t] * upT_n[r, o]
                (hT block stationary, upT moving with free dim 512)
      copyback: DVE adds the (pre-scaled) bias during PSUM->SBUF copy.
  - All weights (downT, upT, bias) stay resident in SBUF.

Wait-slot legalization: this container's walrus accepts at most 1 sync-wait
on a matmul and ~2 on other instructions. The kernel is structured so every
matmul has deps on at most ONE other processor (DVE or one DMA lane):
  - tiny DVE "funnel" copies absorb DMA-completion deps for const tiles,
  - a tiny PE matmul at start absorbs the DVE funnel tick into PE's clock,
  - PSUM pool depths chosen so slot-release deps are covered by older waits,
  - tiny DVE funnel after each output DMA so SBUF slot releases reach DVE.
The Tile tail drain (which waits on every semaphore) is split into a chain
of NoOps carrying <=2 waits each via a TileContext subclass.
"""

from contextlib import ExitStack

import numpy as np

import concourse.bass as bass
import concourse.mybir as mybir
import concourse.tile as tile
from concourse.vector_clock import ScopedClock

WEIGHT = 0.8
N_CORES = 8
B, S, D = 4, 4096, 4096
NL, R, O = 3, 128, 4096
M = B * S                    # 16384 tokens total
T = M // N_CORES             # 2048 tokens per core
NR = NL * R                  # 384
NO = NL * O                  # 12288

P = 128                      # SBUF partitions
TB = 256                     # token block (phase-1 moving free dim)
DO = D // P                  # 32 contraction chunks
OC = 512                     # phase-2 moving free dim / PSUM tile

F32 = mybir.dt.float32
F32R = mybir.dt.float32r
BF16 = mybir.dt.bfloat16

MAX_DRAIN_WAITS = 1


class SplitDrainTileContext(tile.TileContext):
    """TileContext whose tail drain splits its waits into <=2 per instruction.

    The stock tail emits one SP Drain carrying a wait for every live
    semaphore; this walrus build rejects >2 sync-waits per instruction.
    Semantics are identical: all waits still complete before the all-engine
    barrier that follows.
    """

    def _drain_and_barrier(self, tick_clock, wait_clock):
        drain_inst = self.nc.sync.drain()
        wait_clock.add_sem_waits(
            drain_inst.ins, ScopedClock({None: tick_clock.global_clock})
        )
        si = drain_inst.ins.sync_info
        if si is not None and len(si.on_wait) > MAX_DRAIN_WAITS:
            waits = list(si.on_wait)
            drain_inst.ins.sync_info = mybir.SyncInfo(
                on_wait=waits[:MAX_DRAIN_WAITS], on_update=list(si.on_update)
            )
            for i in range(MAX_DRAIN_WAITS, len(waits), MAX_DRAIN_WAITS):
                nop = self.nc.sync.nop(nofuse=True)
                nop.ins.sync_info = mybir.SyncInfo(
                    on_wait=waits[i:i + MAX_DRAIN_WAITS], on_update=[]
                )

        self.nc.all_engine_barrier()
        popped = self.nc._tile_sem_poison_stack.pop()
        assert popped is self._sem_poison
        self.nc.clear_and_free_semaphores(list(self.sems.allocated().values()))
        self.nc.all_engine_barrier()


def build_nc(t_core: int = T) -> bass.Bass:
    assert t_core % TB == 0
    n_tb = t_core // TB

    nc = bass.Bass("TRN2", target_bir_lowering=False, debug=False)

    xT = nc.dram_tensor("xT", [D, t_core], F32R, kind="ExternalInput")
    dT = nc.dram_tensor("dT", [D, NR], F32R, kind="ExternalInput")
    uT = nc.dram_tensor("uT", [R, NO], F32R, kind="ExternalInput")
    bw = nc.dram_tensor("bw", [1, NO], BF16, kind="ExternalInput")
    y = nc.dram_tensor("y", [t_core, NO], F32, kind="ExternalOutput")

    with tile.TileContext(nc) as tc, ExitStack() as ctx:
        const = ctx.enter_context(tc.tile_pool(name="const", bufs=1))
        xpool = ctx.enter_context(tc.tile_pool(name="xpool", bufs=2))
        hpool = ctx.enter_context(tc.tile_pool(name="hpool", bufs=2))
        ypool = ctx.enter_context(tc.tile_pool(name="ypool", bufs=2))
        ps_h = ctx.enter_context(tc.tile_pool(name="ps_h", bufs=6, space="PSUM"))
        ps_y = ctx.enter_context(tc.tile_pool(name="ps_y", bufs=2, space="PSUM"))

        # Resident weights
        dT_sb = const.tile([P, DO, NR], F32R, name="dT_sb")
        nc.sync.dma_start(dT_sb[:], dT.ap().rearrange("(do di) nr -> di do nr", di=P))
        uT_sb = const.tile([P, NO], F32R, name="uT_sb")
        nc.sync.dma_start(uT_sb[:], uT[:, :])
        bw_sb = const.tile([P, NO], BF16, name="bw_sb")
        nc.sync.dma_start(bw_sb[:], bw.ap().to_broadcast((P, NO)))

        xTr = xT.ap().rearrange("(do di) t -> di do t", di=P)
        DH = DO // 2  # d-chunks per x half-load

        for tb in range(n_tb):
            # Load this block's x slice in two halves so MMs start early.
            xts = []
            for h in range(2):
                xt = xpool.tile([P, DH, TB], F32R, tag="xt", name=f"xt{tb}_{h}")
                nc.sync.dma_start(
                    xt[:], xTr[:, h * DH:(h + 1) * DH, tb * TB:(tb + 1) * TB]
                )
                xts.append(xt)

            # Phase 1: hT_n[r, 0:TB] accumulated over all d chunks.
            hps = [
                ps_h.tile([P, TB], F32, tag="hps", name=f"hps{tb}_{n}")
                for n in range(NL)
            ]
            for dc in range(DO):
                xs = xts[dc // DH][:, dc % DH, :]
                for n in range(NL):
                    nc.tensor.matmul(
                        hps[n][:],
                        dT_sb[:, dc, n * R:(n + 1) * R],
                        xs,
                        start=(dc == 0),
                        stop=(dc == DO - 1),
                    )

            hT = hpool.tile([P, NL, TB], F32R, tag="hT", name=f"hT{tb}")
            for n in range(NL):
                nc.vector.tensor_copy(hT[:, n, :], hps[n][:])

            # Phase 2: y[t, o] per 128-token sub-block, per branch, per o half.
            for th in range(TB // P):
                t0 = tb * TB + th * P
                lhs = [hT[:, n, th * P:(th + 1) * P] for n in range(NL)]
                for n in range(NL):
                    o0 = n * O
                    ysb = ypool.tile([P, O], F32, tag="ysb",
                                     name=f"ysb{tb}_{th}_{n}")
                    for oc in range(O // OC):
                        yps = ps_y.tile([P, OC], F32, tag="yps",
                                        name=f"yps{tb}_{th}_{n}_{oc}")
                        nc.tensor.matmul(
                            yps[:],
                            lhs[n],
                            uT_sb[:, o0 + oc * OC: o0 + (oc + 1) * OC],
                            start=True,
                            stop=True,
                        )
                        nc.vector.tensor_add(
                            ysb[:, oc * OC:(oc + 1) * OC],
                            yps[:],
                            bw_sb[:, o0 + oc * OC: o0 + (oc + 1) * OC],
                        )
                    nc.sync.dma_start(y[t0:t0 + P, o0: o0 + O], ysb[:])

    _wrap_to_json_with_wait_split(nc)
    return nc


def _legalize_wait_counts(bir: dict) -> None:
    """Split multi-wait instructions: this walrus accepts only ONE sync-wait
    per instruction. Excess waits move onto NoOps inserted just before the
    instruction on the same engine — identical blocking semantics."""
    n_new = 0
    for fn in bir.get("functions", []):
        for blk in fn.get("blocks", []):
            insts = blk.get("instructions", [])
            out = []
            for inst in insts:
                si = inst.get("sync_info")
                waits = (si or {}).get("on_wait") or []
                if len(waits) > 1:
                    for w in waits[:-1]:
                        nonlocal_name = f"I-waitsplit-{id(inst)}-{n_new}"
                        n_new += 1
                        out.append({
                            "debug": inst.get("debug", 0),
                            "engine": inst["engine"],
                            "ins": [],
                            "name": nonlocal_name,
                            "opcode": "NoOp",
                            "outs": [],
                            "sync_info": {"on_update": [], "on_wait": [w]},
                        })
                    si["on_wait"] = [waits[-1]]
                out.append(inst)
            blk["instructions"] = out


def _wrap_to_json_with_wait_split(nc) -> None:
    import json as _json

    orig = nc.to_json_bytes

    def patched():
        d = _json.loads(orig())
        _legalize_wait_counts(d)
        return _json.dumps(d).encode()

    nc.to_json_bytes = patched


def prep_inputs(x, down, up, bias, scales):
    """Host-side marshalling: transpose + fold scales. Returns per-core in_maps."""
    x = np.asarray(x, dtype=np.float32)
    down = np.asarray(down, dtype=np.float32)
    up = np.asarray(up, dtype=np.float32)
    bias = np.asarray(bias, dtype=np.float32)
    scales = np.asarray(scales, dtype=np.float32)

    ws = (WEIGHT * scales).astype(np.float32)                       # [NL]
    xTf = np.ascontiguousarray(x.reshape(M, D).T)                   # [D, M]
    dTf = np.ascontiguousarray(np.transpose(down, (2, 0, 1)).reshape(D, NR))
    uTf = np.ascontiguousarray(
        np.transpose(up * ws[:, None, None], (2, 0, 1)).reshape(R, NO)
    ).astype(np.float32)
    import ml_dtypes
    bwf = np.ascontiguousarray(
        (bias * ws[:, None]).reshape(1, NO)).astype(ml_dtypes.bfloat16)

    in_maps = []
    for c in range(N_CORES):
        in_maps.append({
            "xT": np.ascontiguousarray(xTf[:, c * T:(c + 1) * T]),
            "dT": dTf,
            "uT": uTf,
            "bw": bwf,
        })
    return in_maps


_CACHED_NC = None


def kernel(x, down, up, bias, scales):
    global _CACHED_NC
    from concourse.bass_utils import run_bass_kernel_spmd

    in_maps = prep_inputs(x, down, up, bias, scales)
    if _CACHED_NC is None:
        _CACHED_NC = build_nc(T)
    res = run_bass_kernel_spmd(_CACHED_NC, in_maps, core_ids=list(range(N_CORES)))
    out = np.concatenate([r["y"] for r in res.results], axis=0)
    return out.reshape(B, S, NO)


# revision 9
# speedup vs baseline: 1.1008x; 1.0390x over previous
"""ConcatenatedLoRALinearSidecarLayer kernel for 8x TRN2 NeuronCores.

Reference computation (per LoRA branch n, then concat over n on the last dim):
    h_n = x @ down_n.T                      # [M, R]
    y_n = (h_n @ up_n.T + bias_n) * (WEIGHT * scales_n)
    out = concat_n(y_n)                     # [M, N*O]

Strategy:
  - Data-parallel over tokens M = B*S = 16384 -> 2048 tokens per core.
  - Host-side prep: transpose x to [D, M] so the device only ever does
    contiguous DMAs; fold WEIGHT*scales into up and bias.
  - Per core, for each 256-token block:
      phase 1:  hT_n[r, t] = sum_d downT_n[d, r] * xT[d, t]
                (downT stationary, xT moving with free dim 256 -> fp32r runs
                 at 1 cycle/row on the PE)
      phase 2:  y[t, o] = sum_r hT_n[r, t] * upT_n[r, o]
                (hT block stationary, upT moving with free dim 512)
      copyback: DVE adds the (pre-scaled) bias during PSUM->SBUF copy.
  - All weights (downT, upT, bias) stay resident in SBUF.

Wait-slot legalization: this container's walrus accepts at most 1 sync-wait
on a matmul and ~2 on other instructions. The kernel is structured so every
matmul has deps on at most ONE other processor (DVE or one DMA lane):
  - tiny DVE "funnel" copies absorb DMA-completion deps for const tiles,
  - a tiny PE matmul at start absorbs the DVE funnel tick into PE's clock,
  - PSUM pool depths chosen so slot-release deps are covered by older waits,
  - tiny DVE funnel after each output DMA so SBUF slot releases reach DVE.
The Tile tail drain (which waits on every semaphore) is split into a chain
of NoOps carrying <=2 waits each via a TileContext subclass.
"""

from contextlib import ExitStack

import numpy as np

import concourse.bass as bass
import concourse.mybir as mybir
import concourse.tile as tile
from concourse.vector_clock import ScopedClock

WEIGHT = 0.8
N_CORES = 8
B, S, D = 4, 4096, 4096
NL, R, O = 3, 128, 4096
M = B * S                    # 16384 tokens total
T = M // N_CORES             # 2048 tokens per core
NR = NL * R                  # 384
NO = NL * O                  # 12288

P = 128                      # SBUF partitions
TB = 256                     # token block (phase-1 moving free dim)
DO = D // P                  # 32 contraction chunks
OC = 512                     # phase-2 moving free dim / PSUM tile

F32 = mybir.dt.float32
F32R = mybir.dt.float32r
BF16 = mybir.dt.bfloat16

MAX_DRAIN_WAITS = 1


class SplitDrainTileContext(tile.TileContext):
    """TileContext whose tail drain splits its waits into <=2 per instruction.

    The stock tail emits one SP Drain carrying a wait for every live
    semaphore; this walrus build rejects >2 sync-waits per instruction.
    Semantics are identical: all waits still complete before the all-engine
    barrier that follows.
    """

    def _drain_and_barrier(self, tick_clock, wait_clock):
        drain_inst = self.nc.sync.drain()
        wait_clock.add_sem_waits(
            drain_inst.ins, ScopedClock({None: tick_clock.global_clock})
        )
        si = drain_inst.ins.sync_info
        if si is not None and len(si.on_wait) > MAX_DRAIN_WAITS:
            waits = list(si.on_wait)
            drain_inst.ins.sync_info = mybir.SyncInfo(
                on_wait=waits[:MAX_DRAIN_WAITS], on_update=list(si.on_update)
            )
            for i in range(MAX_DRAIN_WAITS, len(waits), MAX_DRAIN_WAITS):
                nop = self.nc.sync.nop(nofuse=True)
                nop.ins.sync_info = mybir.SyncInfo(
                    on_wait=waits[i:i + MAX_DRAIN_WAITS], on_update=[]
                )

        self.nc.all_engine_barrier()
        popped = self.nc._tile_sem_poison_stack.pop()
        assert popped is self._sem_poison
        self.nc.clear_and_free_semaphores(list(self.sems.allocated().values()))
        self.nc.all_engine_barrier()


def build_nc(t_core: int = T) -> bass.Bass:
    assert t_core % TB == 0
    n_tb = t_core // TB

    nc = bass.Bass("TRN2", target_bir_lowering=False, debug=False)

    xT = nc.dram_tensor("xT", [D, t_core], F32R, kind="ExternalInput")
    dT = nc.dram_tensor("dT", [D, NR], F32R, kind="ExternalInput")
    uT = nc.dram_tensor("uT", [R, NO], F32R, kind="ExternalInput")
    bw = nc.dram_tensor("bw", [1, NO], BF16, kind="ExternalInput")
    y = nc.dram_tensor("y", [t_core, NO], F32, kind="ExternalOutput")

    with tile.TileContext(nc) as tc, ExitStack() as ctx:
        const = ctx.enter_context(tc.tile_pool(name="const", bufs=1))
        xpool = ctx.enter_context(tc.tile_pool(name="xpool", bufs=2))
        hpool = ctx.enter_context(tc.tile_pool(name="hpool", bufs=2))
        ypool = ctx.enter_context(tc.tile_pool(name="ypool", bufs=3))
        ps_h = ctx.enter_context(tc.tile_pool(name="ps_h", bufs=4, space="PSUM"))
        ps_y = ctx.enter_context(tc.tile_pool(name="ps_y", bufs=4, space="PSUM"))

        # Resident weights
        dT_sb = const.tile([P, DO, NR], F32R, name="dT_sb")
        nc.sync.dma_start(dT_sb[:], dT.ap().rearrange("(do di) nr -> di do nr", di=P))
        uT_sb = const.tile([P, NO], F32R, name="uT_sb")
        nc.sync.dma_start(uT_sb[:], uT[:, :])
        bw_sb = const.tile([P, NO], BF16, name="bw_sb")
        nc.sync.dma_start(bw_sb[:], bw.ap().to_broadcast((P, NO)))

        xTr = xT.ap().rearrange("(do di) t -> di do t", di=P)
        DH = DO // 2  # d-chunks per x half-load

        for tb in range(n_tb):
            # Load this block's x slice in two halves so MMs start early.
            xts = []
            for h in range(2):
                xt = xpool.tile([P, DH, TB], F32R, tag="xt", name=f"xt{tb}_{h}")
                nc.sync.dma_start(
                    xt[:], xTr[:, h * DH:(h + 1) * DH, tb * TB:(tb + 1) * TB]
                )
                xts.append(xt)

            # Phase 1: hT_n[r, 0:TB] accumulated over all d chunks.
            hps = [
                ps_h.tile([P, TB], F32, tag="hps", name=f"hps{tb}_{n}")
                for n in range(NL)
            ]
            for dc in range(DO):
                xs = xts[dc // DH][:, dc % DH, :]
                for n in range(NL):
                    nc.tensor.matmul(
                        hps[n][:],
                        dT_sb[:, dc, n * R:(n + 1) * R],
                        xs,
                        start=(dc == 0),
                        stop=(dc == DO - 1),
                    )

            hT = hpool.tile([P, NL, TB], F32R, tag="hT", name=f"hT{tb}")
            for n in range(NL):
                nc.vector.tensor_copy(hT[:, n, :], hps[n][:])

            # Phase 2: y[t, o] per 128-token sub-block, per branch, per o half.
            for th in range(TB // P):
                t0 = tb * TB + th * P
                lhs = [hT[:, n, th * P:(th + 1) * P] for n in range(NL)]
                for n in range(NL):
                    o0 = n * O
                    ysb = ypool.tile([P, O], F32, tag="ysb",
                                     name=f"ysb{tb}_{th}_{n}")
                    for oc in range(O // OC):
                        yps = ps_y.tile([P, OC], F32, tag="yps",
                                        name=f"yps{tb}_{th}_{n}_{oc}")
                        nc.tensor.matmul(
                            yps[:],
                            lhs[n],
                            uT_sb[:, o0 + oc * OC: o0 + (oc + 1) * OC],
                            start=True,
                            stop=True,
                        )
                        nc.vector.tensor_add(
                            ysb[:, oc * OC:(oc + 1) * OC],
                            yps[:],
                            bw_sb[:, o0 + oc * OC: o0 + (oc + 1) * OC],
                        )
                    nc.sync.dma_start(y[t0:t0 + P, o0: o0 + O], ysb[:])

    _wrap_to_json_with_wait_split(nc)
    return nc


def _legalize_wait_counts(bir: dict) -> None:
    """Split multi-wait instructions: this walrus accepts only ONE sync-wait
    per instruction. Excess waits move onto NoOps inserted just before the
    instruction on the same engine — identical blocking semantics."""
    n_new = 0
    for fn in bir.get("functions", []):
        for blk in fn.get("blocks", []):
            insts = blk.get("instructions", [])
            out = []
            for inst in insts:
                si = inst.get("sync_info")
                waits = (si or {}).get("on_wait") or []
                if len(waits) > 1:
                    for w in waits[:-1]:
                        nonlocal_name = f"I-waitsplit-{id(inst)}-{n_new}"
                        n_new += 1
                        out.append({
                            "debug": inst.get("debug", 0),
                            "engine": inst["engine"],
                            "ins": [],
                            "name": nonlocal_name,
                            "opcode": "NoOp",
                            "outs": [],
                            "sync_info": {"on_update": [], "on_wait": [w]},
                        })
                    si["on_wait"] = [waits[-1]]
                out.append(inst)
            blk["instructions"] = out


def _wrap_to_json_with_wait_split(nc) -> None:
    import json as _json

    orig = nc.to_json_bytes

    def patched():
        d = _json.loads(orig())
        _legalize_wait_counts(d)
        return _json.dumps(d).encode()

    nc.to_json_bytes = patched


def prep_inputs(x, down, up, bias, scales):
    """Host-side marshalling: transpose + fold scales. Returns per-core in_maps."""
    x = np.asarray(x, dtype=np.float32)
    down = np.asarray(down, dtype=np.float32)
    up = np.asarray(up, dtype=np.float32)
    bias = np.asarray(bias, dtype=np.float32)
    scales = np.asarray(scales, dtype=np.float32)

    ws = (WEIGHT * scales).astype(np.float32)                       # [NL]
    xTf = np.ascontiguousarray(x.reshape(M, D).T)                   # [D, M]
    dTf = np.ascontiguousarray(np.transpose(down, (2, 0, 1)).reshape(D, NR))
    uTf = np.ascontiguousarray(
        np.transpose(up * ws[:, None, None], (2, 0, 1)).reshape(R, NO)
    ).astype(np.float32)
    import ml_dtypes
    bwf = np.ascontiguousarray(
        (bias * ws[:, None]).reshape(1, NO)).astype(ml_dtypes.bfloat16)

    in_maps = []
    for c in range(N_CORES):
        in_maps.append({
            "xT": np.ascontiguousarray(xTf[:, c * T:(c + 1) * T]),
            "dT": dTf,
            "uT": uTf,
            "bw": bwf,
        })
    return in_maps


_CACHED_NC = None


def kernel(x, down, up, bias, scales):
    global _CACHED_NC
    from concourse.bass_utils import run_bass_kernel_spmd

    in_maps = prep_inputs(x, down, up, bias, scales)
    if _CACHED_NC is None:
        _CACHED_NC = build_nc(T)
    res = run_bass_kernel_spmd(_CACHED_NC, in_maps, core_ids=list(range(N_CORES)))
    out = np.concatenate([r["y"] for r in res.results], axis=0)
    return out.reshape(B, S, NO)


# revision 10
# speedup vs baseline: 1.1109x; 1.0091x over previous
"""ConcatenatedLoRALinearSidecarLayer kernel for 8x TRN2 NeuronCores.

Reference computation (per LoRA branch n, then concat over n on the last dim):
    h_n = x @ down_n.T                      # [M, R]
    y_n = (h_n @ up_n.T + bias_n) * (WEIGHT * scales_n)
    out = concat_n(y_n)                     # [M, N*O]

Strategy:
  - Data-parallel over tokens M = B*S = 16384 -> 2048 tokens per core.
  - Host-side prep: transpose x to [D, M] so the device only ever does
    contiguous DMAs; fold WEIGHT*scales into up and bias.
  - Per core, for each 256-token block:
      phase 1:  hT_n[r, t] = sum_d downT_n[d, r] * xT[d, t]
                (downT stationary, xT moving with free dim 256 -> fp32r runs
                 at 1 cycle/row on the PE)
      phase 2:  y[t, o] = sum_r hT_n[r, t] * upT_n[r, o]
                (hT block stationary, upT moving with free dim 512)
      copyback: DVE adds the (pre-scaled) bias during PSUM->SBUF copy.
  - All weights (downT, upT, bias) stay resident in SBUF.

Wait-slot legalization: this container's walrus accepts at most 1 sync-wait
on a matmul and ~2 on other instructions. The kernel is structured so every
matmul has deps on at most ONE other processor (DVE or one DMA lane):
  - tiny DVE "funnel" copies absorb DMA-completion deps for const tiles,
  - a tiny PE matmul at start absorbs the DVE funnel tick into PE's clock,
  - PSUM pool depths chosen so slot-release deps are covered by older waits,
  - tiny DVE funnel after each output DMA so SBUF slot releases reach DVE.
The Tile tail drain (which waits on every semaphore) is split into a chain
of NoOps carrying <=2 waits each via a TileContext subclass.
"""

from contextlib import ExitStack

import numpy as np

import concourse.bass as bass
import concourse.mybir as mybir
import concourse.tile as tile
from concourse.vector_clock import ScopedClock

WEIGHT = 0.8
N_CORES = 8
B, S, D = 4, 4096, 4096
NL, R, O = 3, 128, 4096
M = B * S                    # 16384 tokens total
T = M // N_CORES             # 2048 tokens per core
NR = NL * R                  # 384
NO = NL * O                  # 12288

P = 128                      # SBUF partitions
TB = 256                     # token block (phase-1 moving free dim)
DO = D // P                  # 32 contraction chunks
OC = 512                     # phase-2 moving free dim / PSUM tile

F32 = mybir.dt.float32
F32R = mybir.dt.float32r
BF16 = mybir.dt.bfloat16

MAX_DRAIN_WAITS = 1

# Phase-1 (x @ down^T) operand dtype: bf16 halves the x/down DMA traffic at
# the cost of ~1e-3 relative error (vs ~2.5e-4 with fp32r everywhere).
PHASE1_BF16 = False


class SplitDrainTileContext(tile.TileContext):
    """TileContext whose tail drain splits its waits into <=2 per instruction.

    The stock tail emits one SP Drain carrying a wait for every live
    semaphore; this walrus build rejects >2 sync-waits per instruction.
    Semantics are identical: all waits still complete before the all-engine
    barrier that follows.
    """

    def _drain_and_barrier(self, tick_clock, wait_clock):
        drain_inst = self.nc.sync.drain()
        wait_clock.add_sem_waits(
            drain_inst.ins, ScopedClock({None: tick_clock.global_clock})
        )
        si = drain_inst.ins.sync_info
        if si is not None and len(si.on_wait) > MAX_DRAIN_WAITS:
            waits = list(si.on_wait)
            drain_inst.ins.sync_info = mybir.SyncInfo(
                on_wait=waits[:MAX_DRAIN_WAITS], on_update=list(si.on_update)
            )
            for i in range(MAX_DRAIN_WAITS, len(waits), MAX_DRAIN_WAITS):
                nop = self.nc.sync.nop(nofuse=True)
                nop.ins.sync_info = mybir.SyncInfo(
                    on_wait=waits[i:i + MAX_DRAIN_WAITS], on_update=[]
                )

        self.nc.all_engine_barrier()
        popped = self.nc._tile_sem_poison_stack.pop()
        assert popped is self._sem_poison
        self.nc.clear_and_free_semaphores(list(self.sems.allocated().values()))
        self.nc.all_engine_barrier()


def build_nc(t_core: int = T) -> bass.Bass:
    assert t_core % TB == 0
    n_tb = t_core // TB

    nc = bass.Bass("TRN2", target_bir_lowering=False, debug=False)

    p1dt = BF16 if PHASE1_BF16 else F32R
    xT = nc.dram_tensor("xT", [D, t_core], p1dt, kind="ExternalInput")
    dT = nc.dram_tensor("dT", [D, NR], p1dt, kind="ExternalInput")
    uT = nc.dram_tensor("uT", [R, NO], F32R, kind="ExternalInput")
    bw = nc.dram_tensor("bw", [1, NO], BF16, kind="ExternalInput")
    y = nc.dram_tensor("y", [t_core, NO], F32, kind="ExternalOutput")

    with tile.TileContext(nc) as tc, ExitStack() as ctx:
        const = ctx.enter_context(tc.tile_pool(name="const", bufs=1))
        xpool = ctx.enter_context(tc.tile_pool(name="xpool", bufs=2))
        hpool = ctx.enter_context(tc.tile_pool(name="hpool", bufs=2))
        ypool = ctx.enter_context(tc.tile_pool(name="ypool", bufs=3))
        ps_h = ctx.enter_context(tc.tile_pool(name="ps_h", bufs=4, space="PSUM"))
        ps_y = ctx.enter_context(tc.tile_pool(name="ps_y", bufs=4, space="PSUM"))

        # Resident weights
        dT_sb = const.tile([P, DO, NR], p1dt, name="dT_sb")
        nc.sync.dma_start(dT_sb[:], dT.ap().rearrange("(do di) nr -> di do nr", di=P))
        uT_sb = const.tile([P, NO], F32R, name="uT_sb")
        nc.sync.dma_start(uT_sb[:], uT[:, :])
        bw_sb = const.tile([P, NO], BF16, name="bw_sb")
        nc.sync.dma_start(bw_sb[:], bw.ap().to_broadcast((P, NO)))

        xTr = xT.ap().rearrange("(do di) t -> di do t", di=P)
        DH = DO // 2  # d-chunks per x half-load

        for tb in range(n_tb):
            # Load this block's x slice in two halves so MMs start early.
            xts = []
            for h in range(2):
                xt = xpool.tile([P, DH, TB], p1dt, tag="xt", name=f"xt{tb}_{h}")
                nc.sync.dma_start(
                    xt[:], xTr[:, h * DH:(h + 1) * DH, tb * TB:(tb + 1) * TB]
                )
                xts.append(xt)

            # Phase 1: hT_n[r, 0:TB] accumulated over all d chunks.
            hps = [
                ps_h.tile([P, TB], F32, tag="hps", name=f"hps{tb}_{n}")
                for n in range(NL)
            ]
            for dc in range(DO):
                xs = xts[dc // DH][:, dc % DH, :]
                for n in range(NL):
                    nc.tensor.matmul(
                        hps[n][:],
                        dT_sb[:, dc, n * R:(n + 1) * R],
                        xs,
                        start=(dc == 0),
                        stop=(dc == DO - 1),
                    )

            hT = hpool.tile([P, NL, TB], F32R, tag="hT", name=f"hT{tb}")
            for n in range(NL):
                nc.vector.tensor_copy(hT[:, n, :], hps[n][:])

            # Phase 2: y[t, o] per 128-token sub-block, per branch, per o half.
            for th in range(TB // P):
                t0 = tb * TB + th * P
                lhs = [hT[:, n, th * P:(th + 1) * P] for n in range(NL)]
                for n in range(NL):
                    o0 = n * O
                    ysb = ypool.tile([P, O], F32, tag="ysb",
                                     name=f"ysb{tb}_{th}_{n}")
                    for oc in range(O // OC):
                        yps = ps_y.tile([P, OC], F32, tag="yps",
                                        name=f"yps{tb}_{th}_{n}_{oc}")
                        nc.tensor.matmul(
                            yps[:],
                            lhs[n],
                            uT_sb[:, o0 + oc * OC: o0 + (oc + 1) * OC],
                            start=True,
                            stop=True,
                        )
                        nc.vector.tensor_add(
                            ysb[:, oc * OC:(oc + 1) * OC],
                            yps[:],
                            bw_sb[:, o0 + oc * OC: o0 + (oc + 1) * OC],
                        )
                    nc.sync.dma_start(y[t0:t0 + P, o0: o0 + O], ysb[:])

    _wrap_to_json_with_wait_split(nc)
    return nc


def _legalize_wait_counts(bir: dict) -> None:
    """Split multi-wait instructions: this walrus accepts only ONE sync-wait
    per instruction. Excess waits move onto NoOps inserted just before the
    instruction on the same engine — identical blocking semantics."""
    n_new = 0
    for fn in bir.get("functions", []):
        for blk in fn.get("blocks", []):
            insts = blk.get("instructions", [])
            out = []
            for inst in insts:
                si = inst.get("sync_info")
                waits = (si or {}).get("on_wait") or []
                if len(waits) > 1:
                    for w in waits[:-1]:
                        nonlocal_name = f"I-waitsplit-{id(inst)}-{n_new}"
                        n_new += 1
                        out.append({
                            "debug": inst.get("debug", 0),
                            "engine": inst["engine"],
                            "ins": [],
                            "name": nonlocal_name,
                            "opcode": "NoOp",
                            "outs": [],
                            "sync_info": {"on_update": [], "on_wait": [w]},
                        })
                    si["on_wait"] = [waits[-1]]
                out.append(inst)
            blk["instructions"] = out


def _wrap_to_json_with_wait_split(nc) -> None:
    import json as _json

    orig = nc.to_json_bytes

    def patched():
        d = _json.loads(orig())
        _legalize_wait_counts(d)
        return _json.dumps(d).encode()

    nc.to_json_bytes = patched


def prep_inputs(x, down, up, bias, scales):
    """Host-side marshalling: transpose + fold scales. Returns per-core in_maps."""
    x = np.asarray(x, dtype=np.float32)
    down = np.asarray(down, dtype=np.float32)
    up = np.asarray(up, dtype=np.float32)
    bias = np.asarray(bias, dtype=np.float32)
    scales = np.asarray(scales, dtype=np.float32)

    import ml_dtypes
    p1np = ml_dtypes.bfloat16 if PHASE1_BF16 else np.float32
    ws = (WEIGHT * scales).astype(np.float32)                       # [NL]
    xTf = np.ascontiguousarray(x.reshape(M, D).T).astype(p1np)      # [D, M]
    dTf = np.ascontiguousarray(
        np.transpose(down, (2, 0, 1)).reshape(D, NR)).astype(p1np)
    uTf = np.ascontiguousarray(
        np.transpose(up * ws[:, None, None], (2, 0, 1)).reshape(R, NO)
    ).astype(np.float32)
    import ml_dtypes
    bwf = np.ascontiguousarray(
        (bias * ws[:, None]).reshape(1, NO)).astype(ml_dtypes.bfloat16)

    in_maps = []
    for c in range(N_CORES):
        in_maps.append({
            "xT": np.ascontiguousarray(xTf[:, c * T:(c + 1) * T]),
            "dT": dTf,
            "uT": uTf,
            "bw": bwf,
        })
    return in_maps


_CACHED_NC = None


def kernel(x, down, up, bias, scales):
    global _CACHED_NC
    from concourse.bass_utils import run_bass_kernel_spmd

    in_maps = prep_inputs(x, down, up, bias, scales)
    if _CACHED_NC is None:
        _CACHED_NC = build_nc(T)
    res = run_bass_kernel_spmd(_CACHED_NC, in_maps, core_ids=list(range(N_CORES)))
    out = np.concatenate([r["y"] for r in res.results], axis=0)
    return out.reshape(B, S, NO)
